# revision 8
# baseline (speedup 1.0000x reference)
"""Trainium2 Bass kernel: 12-head attention (B=2, N=2048, C=768) on 8 NeuronCores.

Sharding: core c -> batch b = c // 4, head-group g = c % 4 (heads 3g..3g+2).
Per core: column-sharded QKV projection, head-sharded attention, AllGather of
normalized attention outputs (channel-major) within each batch's 4-core group,
then the output projection.

Device layouts are "transposed" (channel-major, [C, tokens]) so that
 - the exp mask bias is a per-partition (key) ACT bias,
 - the AllGather concat axis (partitions) is the head/channel axis,
 - the projection consumes the gathered tensor directly as matmul rhs.
The softmax denominator is obtained via an extra all-ones column appended to V
(so P@[V*m, m] yields both the weighted values and the row sums in one pass),
and the division is applied via vector reciprocal + gpsimd partition_broadcast.
"""

import numpy as np
import ml_dtypes

B, N, C = 2, 2048, 768
H, HD = 12, 64
HPG = 3            # heads per core
GPB = 4            # cores (head-groups) per batch
NCORES = 8
SCALE = float(HD) ** -0.5
MASK_NEG = -50.0
KCH = N // 128     # 16 key chunks
DCH = C // 128     # 6 contraction chunks

bf = ml_dtypes.bfloat16

_cache = {}


def _build():
    import concourse.bass as bass
    import concourse.mybir as mybir
    import concourse.tile as tile
    from concourse import bacc

    fp32 = mybir.dt.float32
    bfl = mybir.dt.bfloat16
    EXP = mybir.ActivationFunctionType.Exp
    MULT = mybir.AluOpType.mult

    nc = bacc.Bacc(None, num_devices=NCORES)
    xT = nc.declare_dram_parameter("xT", [C, N], bfl, isOutput=False)
    wqk = nc.declare_dram_parameter("wqk", [C, 2 * HPG * HD], bfl, isOutput=False)
    wv = nc.declare_dram_parameter("wv", [C, HPG * HD], bfl, isOutput=False)
    wp = nc.declare_dram_parameter("wp", [C, C], bfl, isOutput=False)
    bp = nc.declare_dram_parameter("bp", [128, DCH], fp32, isOutput=False)
    mb = nc.declare_dram_parameter("mb", [128, KCH], fp32, isOutput=False)
    mf = nc.declare_dram_parameter("mf", [128, KCH], fp32, isOutput=False)
    out = nc.declare_dram_parameter("out", [C, N], fp32, isOutput=True)

    with tile.TileContext(nc) as tc:
        with (
            tc.tile_pool(name="const", bufs=1) as cpool,
            tc.tile_pool(name="work", bufs=1) as wpool,
            tc.tile_pool(name="pp", bufs=3) as ppool,
            tc.tile_pool(name="dram", bufs=1, space="DRAM") as dpool,
        ):
            # ---------------- input loads ----------------
            xT_sb = cpool.tile([128, DCH, N], bfl, tag="xT")
            nc.sync.dma_start(xT_sb[:], xT.rearrange("(o p) t -> p o t", p=128))
            wqk_sb = cpool.tile([128, DCH, 2 * HPG * HD], bfl, tag="wqk")
            nc.sync.dma_start(wqk_sb[:], wqk.rearrange("(o p) c -> p o c", p=128))
            wv_sb = cpool.tile([128, DCH, HPG * HD], bfl, tag="wv")
            nc.sync.dma_start(wv_sb[:], wv.rearrange("(o p) c -> p o c", p=128))
            wp_sb = cpool.tile([128, DCH, C], bfl, tag="wp")
            nc.sync.dma_start(wp_sb[:], wp.rearrange("(o p) c -> p o c", p=128))
            bp_sb = cpool.tile([128, DCH], fp32, tag="bp")
            nc.sync.dma_start(bp_sb[:], bp[:])
            mb_sb = cpool.tile([128, KCH], fp32, tag="mb")
            nc.sync.dma_start(mb_sb[:], mb[:])
            mf_sb = cpool.tile([128, KCH], fp32, tag="mf")
            nc.sync.dma_start(mf_sb[:], mf[:])

            qT = wpool.tile([64, HPG, N], bfl, tag="qT")
            kT = wpool.tile([64, HPG, N], bfl, tag="kT")
            V3 = wpool.tile([128, KCH, HPG, HD + 1], bfl, tag="V3")

            # ---------------- QK^T projection (channel-major) ----------------
            # wqk columns: [q_h0 q_h1 q_h2 k_h0 k_h1 k_h2], 64 each.
            dests = [(qT, 0), (qT, 1), (qT, 2), (kT, 0), (kT, 1), (kT, 2)]
            qkps_cm = tc.tile_pool(name="qkps", bufs=2, space="PSUM")
            vps_cm = tc.tile_pool(name="vps", bufs=2, space="PSUM")
            qkps = qkps_cm.__enter__(); vps = vps_cm.__enter__()
            for m in range(3):
                for nt in range(2):
                    qk_t = qkps.tile([128, N // 2], fp32, tag="qk")
                    for n2 in range(2):
                        for kk in range(DCH):
                            nc.tensor.matmul(
                                qk_t[:, n2 * 512 : (n2 + 1) * 512],
                                lhsT=wqk_sb[:, kk, m * 128 : (m + 1) * 128],
                                rhs=xT_sb[
                                    :, kk, nt * 1024 + n2 * 512 : nt * 1024 + (n2 + 1) * 512
                                ],
                                start=(kk == 0),
                                stop=(kk == DCH - 1),
                            )
                    for half in range(2):
                        dtile, j = dests[m * 2 + half]
                        nc.vector.tensor_copy(
                            dtile[:, j, nt * 1024 : (nt + 1) * 1024],
                            qk_t[half * 64 : (half + 1) * 64, :],
                        )

            # ---------------- V projection (token-major) + mask fold ----------------
            for i in range(KCH):
                v_t = vps.tile([128, HPG * HD], fp32, tag="v")
                for kk in range(DCH):
                    nc.tensor.matmul(
                        v_t[:],
                        lhsT=xT_sb[:, kk, i * 128 : (i + 1) * 128],
                        rhs=wv_sb[:, kk, :],
                        start=(kk == 0),
                        stop=(kk == DCH - 1),
                    )
                nc.vector.tensor_scalar_mul(
                    V3[:, i, :, 0:HD],
                    v_t[:].rearrange("p (h d) -> p h d", h=HPG),
                    mf_sb[:, i : i + 1],
                )
                nc.vector.tensor_copy(
                    V3[:, i, :, HD],
                    mf_sb[:, i : i + 1].to_broadcast((128, HPG)),
                )

            vps_cm.__exit__(None, None, None); qkps_cm.__exit__(None, None, None)

            # ---------------- attention per head ----------------
            sps_cm = tc.tile_pool(name="sps", bufs=2, space="PSUM")
            ops_cm = tc.tile_pool(name="ops", bufs=1, space="PSUM")
            sps = sps_cm.__enter__(); ops = ops_cm.__enter__()
            OnA = wpool.tile([128, N], bfl, tag="OnA")
            OnB = wpool.tile([64, N], bfl, tag="OnB")
            for h in range(HPG):
                o_t = ops.tile([HD + 1, N], fp32, tag="o")
                for i in range(KCH):
                    for qh in range(2):
                        s_t = sps.tile([128, N // 2], fp32, tag="s")
                        for n2 in range(2):
                            nc.tensor.matmul(
                                s_t[:, n2 * 512 : (n2 + 1) * 512],
                                lhsT=kT[:, h, i * 128 : (i + 1) * 128],
                                rhs=qT[:, h, qh * 1024 + n2 * 512 : qh * 1024 + (n2 + 1) * 512],
                                start=True,
                                stop=True,
                            )
                        p_t = ppool.tile([128, N // 2], bfl, tag="p")
                        nc.scalar.activation(
                            p_t[:], s_t[:], EXP, bias=mb_sb[:, i : i + 1], scale=SCALE
                        )
                        for n2 in range(2):
                            qt = qh * 2 + n2
                            nc.tensor.matmul(
                                o_t[:, qt * 512 : (qt + 1) * 512],
                                lhsT=V3[:, i, h, :],
                                rhs=p_t[:, n2 * 512 : (n2 + 1) * 512],
                                start=(i == 0),
                                stop=(i == KCH - 1),
                            )
                # normalization: rows 0..63 = unnormalized O^T, row 64 = sums
                sums = wpool.tile([1, N], fp32, tag="sums")
                nc.scalar.copy(sums[:], o_t[HD : HD + 1, :])
                rinv = wpool.tile([1, N], fp32, tag="rinv")
                nc.vector.reciprocal(rinv[:], sums[:])
                rb = wpool.tile([HD, N], fp32, tag="rb")
                nc.gpsimd.partition_broadcast(rb[:], rinv[:])
                dst = OnA[h * 64 : (h + 1) * 64, :] if h < 2 else OnB[:, :]
                nc.vector.tensor_tensor(dst, o_t[0:HD, :], rb[:], MULT)

            ops_cm.__exit__(None, None, None); sps_cm.__exit__(None, None, None)

            # ---------------- AllGather (per-batch 4-core group) ----------------
            ag_in = dpool.tile([HPG * HD, N], bfl, tag="agin")
            ag_out = dpool.tile([GPB * HPG * HD, N], bfl, tag="agout")
            nc.sync.dma_start(ag_in[0:128, :], OnA[:])
            nc.sync.dma_start(ag_in[128:192, :], OnB[:])
            nc.gpsimd.collective_compute(
                "AllGather",
                mybir.AluOpType.bypass,
                replica_groups=[[0, 1, 2, 3], [4, 5, 6, 7]],
                ins=[ag_in[:].opt()],
                outs=[ag_out[:].opt()],
            )

            # ---------------- output projection ----------------
            pjps_cm = tc.tile_pool(name="pjps", bufs=2, space="PSUM")
            pjps = pjps_cm.__enter__()
            ag_out_t = ag_out.rearrange("(o p) t -> p o t", p=128)
            out_t = out.rearrange("(o p) t -> p o t", p=128)
            for nt in range(4):
                at_sb = ppool.tile([128, DCH, 512], bfl, tag="at")
                nc.sync.dma_start(at_sb[:], ag_out_t[:, :, nt * 512 : (nt + 1) * 512])
                for m in range(DCH):
                    y_ps = pjps.tile([128, 512], fp32, tag="yps")
                    for kk in range(DCH):
                        nc.tensor.matmul(
                            y_ps[:],
                            lhsT=wp_sb[:, kk, m * 128 : (m + 1) * 128],
                            rhs=at_sb[:, kk, :],
                            start=(kk == 0),
                            stop=(kk == DCH - 1),
                        )
                    y_sb = ppool.tile([128, 512], fp32, tag="y")
                    nc.scalar.add(y_sb[:], y_ps[:], bp_sb[:, m : m + 1])
                    nc.sync.dma_start(out_t[:, m, nt * 512 : (nt + 1) * 512], y_sb[:])
            pjps_cm.__exit__(None, None, None)

    nc.finalize()
    return nc


def _shard_inputs(x, mask, w_qkv, w_proj, b_proj):
    in_maps = []
    for c in range(NCORES):
        b, g = c // GPB, c % GPB
        heads = [3 * g, 3 * g + 1, 3 * g + 2]
        qk_cols = [h * HD + d for h in heads for d in range(HD)] + [
            C + h * HD + d for h in heads for d in range(HD)
        ]
        v_cols = [2 * C + h * HD + d for h in heads for d in range(HD)]
        mrow = mask[b].astype(np.float32)
        in_maps.append(
            {
                "xT": np.ascontiguousarray(x[b].T).astype(bf),
                "wqk": np.ascontiguousarray(w_qkv[:, qk_cols]).astype(bf),
                "wv": np.ascontiguousarray(w_qkv[:, v_cols]).astype(bf),
                "wp": w_proj.astype(bf),
                "bp": np.ascontiguousarray(
                    b_proj.astype(np.float32).reshape(DCH, 128).T
                ),
                "mb": np.ascontiguousarray(
                    np.where(mrow > 0.5, 0.0, MASK_NEG).astype(np.float32).reshape(KCH, 128).T
                ),
                "mf": np.ascontiguousarray(mrow.reshape(KCH, 128).T),
            }
        )
    return in_maps


def kernel(x, mask, w_qkv, w_proj, b_proj, _trace=False):
    from concourse.bass_utils import run_bass_kernel_spmd

    if "nc" not in _cache:
        _cache["nc"] = _build()
    nc = _cache["nc"]
    in_maps = _shard_inputs(x, mask, w_qkv, w_proj, b_proj)
    res = run_bass_kernel_spmd(nc, in_maps, core_ids=list(range(NCORES)), trace=_trace)
    y = np.empty((B, N, C), dtype=np.float32)
    for b in range(B):
        y[b] = np.asarray(res.results[GPB * b]["out"]).T
    if _trace:
        _cache["last_exec_time_ns"] = res.exec_time_ns
        _cache["last_profile"] = res.profile_json
    return y


# revision 9
# speedup vs baseline: 1.0964x; 1.0964x over previous
"""Trainium2 Bass kernel: 12-head attention (B=2, N=2048, C=768) on 8 NeuronCores.

Sharding: core c -> batch b = c // 4, head-group g = c % 4 (heads 3g..3g+2).
Per core: column-sharded QKV projection, head-sharded attention, AllGather of
normalized attention outputs (channel-major) within each batch's 4-core group,
then the output projection.

Device layouts are "transposed" (channel-major, [C, tokens]) so that
 - the exp mask bias is a per-partition (key) ACT bias,
 - the AllGather concat axis (partitions) is the head/channel axis,
 - the projection consumes the gathered tensor directly as matmul rhs.
The softmax denominator is obtained via an extra all-ones column appended to V
(so P@[V*m, m] yields both the weighted values and the row sums in one pass),
and the division is applied via vector reciprocal + gpsimd partition_broadcast.
"""

import numpy as np
import ml_dtypes

B, N, C = 2, 2048, 768
H, HD = 12, 64
HPG = 3            # heads per core
GPB = 4            # cores (head-groups) per batch
NCORES = 8
SCALE = float(HD) ** -0.5
MASK_NEG = -50.0
KCH = N // 128     # 16 key chunks
DCH = C // 128     # 6 contraction chunks

bf = ml_dtypes.bfloat16

_cache = {}


def _build():
    import concourse.bass as bass
    import concourse.mybir as mybir
    import concourse.tile as tile
    from concourse import bacc

    fp32 = mybir.dt.float32
    bfl = mybir.dt.bfloat16
    EXP = mybir.ActivationFunctionType.Exp
    MULT = mybir.AluOpType.mult

    nc = bacc.Bacc(None, num_devices=NCORES)
    xT = nc.declare_dram_parameter("xT", [C, N], bfl, isOutput=False)
    wqk = nc.declare_dram_parameter("wqk", [C, 2 * HPG * HD], bfl, isOutput=False)
    wv = nc.declare_dram_parameter("wv", [C, HPG * HD], bfl, isOutput=False)
    wp = nc.declare_dram_parameter("wp", [C, C], bfl, isOutput=False)
    bp = nc.declare_dram_parameter("bp", [1, C], bfl, isOutput=False)
    mb = nc.declare_dram_parameter("mb", [128, KCH], fp32, isOutput=False)
    mf = nc.declare_dram_parameter("mf", [128, KCH], fp32, isOutput=False)
    out = nc.declare_dram_parameter("out", [C, N], fp32, isOutput=True)

    with tile.TileContext(nc) as tc:
        with (
            tc.tile_pool(name="const", bufs=1) as cpool,
            tc.tile_pool(name="work", bufs=1) as wpool,
            tc.tile_pool(name="pp", bufs=3) as ppool,
            tc.tile_pool(name="dram", bufs=1, space="DRAM") as dpool,
        ):
            # ---------------- input loads ----------------
            xT_sb = cpool.tile([128, DCH, N], bfl, tag="xT")
            nc.sync.dma_start(xT_sb[:], xT.rearrange("(o p) t -> p o t", p=128))
            wqk_sb = cpool.tile([128, DCH, 2 * HPG * HD], bfl, tag="wqk")
            nc.sync.dma_start(wqk_sb[:], wqk.rearrange("(o p) c -> p o c", p=128))
            wv_sb = cpool.tile([128, DCH, HPG * HD], bfl, tag="wv")
            nc.sync.dma_start(wv_sb[:], wv.rearrange("(o p) c -> p o c", p=128))
            wp_sb = cpool.tile([128, DCH, C], bfl, tag="wp")
            nc.sync.dma_start(wp_sb[:], wp.rearrange("(o p) c -> p o c", p=128))
            bp_sb = cpool.tile([1, C], bfl, tag="bp")
            nc.sync.dma_start(bp_sb[:], bp[:])
            ones_sb = cpool.tile([1, 512], bfl, tag="ones")
            nc.vector.memset(ones_sb[:], 1.0)
            mb_sb = cpool.tile([128, KCH], fp32, tag="mb")
            nc.sync.dma_start(mb_sb[:], mb[:])
            mf_sb = cpool.tile([128, KCH], fp32, tag="mf")
            nc.sync.dma_start(mf_sb[:], mf[:])

            qT = wpool.tile([64, HPG, N], bfl, tag="qT")
            kT = wpool.tile([64, HPG, N], bfl, tag="kT")
            V3 = wpool.tile([128, KCH, HPG, HD + 1], bfl, tag="V3")

            # ---------------- QK^T projection (channel-major) ----------------
            # wqk columns: [q_h0 q_h1 q_h2 k_h0 k_h1 k_h2], 64 each.
            dests = [(qT, 0), (qT, 1), (qT, 2), (kT, 0), (kT, 1), (kT, 2)]
            qkps_cm = tc.tile_pool(name="qkps", bufs=2, space="PSUM")
            vps_cm = tc.tile_pool(name="vps", bufs=2, space="PSUM")
            qkps = qkps_cm.__enter__(); vps = vps_cm.__enter__()
            for m in range(3):
                for nt in range(2):
                    qk_t = qkps.tile([128, N // 2], fp32, tag="qk")
                    for n2 in range(2):
                        for kk in range(DCH):
                            nc.tensor.matmul(
                                qk_t[:, n2 * 512 : (n2 + 1) * 512],
                                lhsT=wqk_sb[:, kk, m * 128 : (m + 1) * 128],
                                rhs=xT_sb[
                                    :, kk, nt * 1024 + n2 * 512 : nt * 1024 + (n2 + 1) * 512
                                ],
                                start=(kk == 0),
                                stop=(kk == DCH - 1),
                            )
                    for half in range(2):
                        dtile, j = dests[m * 2 + half]
                        nc.vector.tensor_copy(
                            dtile[:, j, nt * 1024 : (nt + 1) * 1024],
                            qk_t[half * 64 : (half + 1) * 64, :],
                        )

            # ---------------- V projection (token-major) + mask fold ----------------
            for i in range(KCH):
                v_t = vps.tile([128, HPG * HD], fp32, tag="v")
                for kk in range(DCH):
                    nc.tensor.matmul(
                        v_t[:],
                        lhsT=xT_sb[:, kk, i * 128 : (i + 1) * 128],
                        rhs=wv_sb[:, kk, :],
                        start=(kk == 0),
                        stop=(kk == DCH - 1),
                    )
                nc.vector.tensor_scalar_mul(
                    V3[:, i, :, 0:HD],
                    v_t[:].rearrange("p (h d) -> p h d", h=HPG),
                    mf_sb[:, i : i + 1],
                )
                nc.vector.tensor_copy(
                    V3[:, i, :, HD],
                    mf_sb[:, i : i + 1].to_broadcast((128, HPG)),
                )

            vps_cm.__exit__(None, None, None); qkps_cm.__exit__(None, None, None)

            # ---------------- attention per head ----------------
            sps_cm = tc.tile_pool(name="sps", bufs=2, space="PSUM")
            ops_cm = tc.tile_pool(name="ops", bufs=1, space="PSUM")
            sps = sps_cm.__enter__(); ops = ops_cm.__enter__()
            OnA = wpool.tile([128, N], bfl, tag="OnA")
            OnB = wpool.tile([64, N], bfl, tag="OnB")
            for h in range(HPG):
                o_t = ops.tile([HD + 1, N], fp32, tag="o")
                for i in range(KCH):
                    for qh in range(2):
                        s_t = sps.tile([128, N // 2], fp32, tag="s")
                        for n2 in range(2):
                            nc.tensor.matmul(
                                s_t[:, n2 * 512 : (n2 + 1) * 512],
                                lhsT=kT[:, h, i * 128 : (i + 1) * 128],
                                rhs=qT[:, h, qh * 1024 + n2 * 512 : qh * 1024 + (n2 + 1) * 512],
                                start=True,
                                stop=True,
                            )
                        p_t = ppool.tile([128, N // 2], bfl, tag="p")
                        nc.scalar.activation(
                            p_t[:], s_t[:], EXP, bias=mb_sb[:, i : i + 1], scale=SCALE
                        )
                        for n2 in range(2):
                            qt = qh * 2 + n2
                            nc.tensor.matmul(
                                o_t[:, qt * 512 : (qt + 1) * 512],
                                lhsT=V3[:, i, h, :],
                                rhs=p_t[:, n2 * 512 : (n2 + 1) * 512],
                                start=(i == 0),
                                stop=(i == KCH - 1),
                            )
                # normalization: rows 0..63 = unnormalized O^T, row 64 = sums
                sums = wpool.tile([1, N], fp32, tag="sums")
                nc.scalar.copy(sums[:], o_t[HD : HD + 1, :])
                o_raw = wpool.tile([HD, N], fp32, tag=f"oraw{h}")
                nc.vector.tensor_copy(o_raw[:], o_t[0:HD, :])
                rbraw = wpool.tile([HD, N], fp32, tag="rbraw")
                nc.gpsimd.partition_broadcast(rbraw[:], sums[:])
                rb = wpool.tile([HD, N], fp32, tag="rb")
                nc.vector.reciprocal(rb[:], rbraw[:])
                dst = OnA[h * 64 : (h + 1) * 64, :] if h < 2 else OnB[:, :]
                nc.vector.tensor_tensor(dst, o_raw[:], rb[:], MULT)

            ops_cm.__exit__(None, None, None); sps_cm.__exit__(None, None, None)

            # ---------------- AllGather (per-batch 4-core group) ----------------
            ag_in = dpool.tile([HPG * HD, N], bfl, tag="agin")
            ag_out = dpool.tile([GPB * HPG * HD, N], bfl, tag="agout")
            nc.sync.dma_start(ag_in[0:128, :], OnA[:])
            nc.sync.dma_start(ag_in[128:192, :], OnB[:])
            nc.gpsimd.collective_compute(
                "AllGather",
                mybir.AluOpType.bypass,
                replica_groups=[[0, 1, 2, 3], [4, 5, 6, 7]],
                ins=[ag_in[:].opt()],
                outs=[ag_out[:].opt()],
            )

            # ---------------- output projection ----------------
            pjps_cm = tc.tile_pool(name="pjps", bufs=2, space="PSUM")
            pjps = pjps_cm.__enter__()
            ag_out_t = ag_out.rearrange("(o p) t -> p o t", p=128)
            out_t = out.rearrange("(o p) t -> p o t", p=128)
            for nt in range(4):
                at_sb = ppool.tile([128, DCH, 512], bfl, tag="at")
                nc.sync.dma_start(at_sb[:], ag_out_t[:, :, nt * 512 : (nt + 1) * 512])
                for m in range(DCH):
                    y_ps = pjps.tile([128, 512], fp32, tag="yps")
                    for kk in range(DCH):
                        nc.tensor.matmul(
                            y_ps[:],
                            lhsT=wp_sb[:, kk, m * 128 : (m + 1) * 128],
                            rhs=at_sb[:, kk, :],
                            start=(kk == 0),
                            stop=False,
                        )
                    nc.tensor.matmul(
                        y_ps[:],
                        lhsT=bp_sb[:, m * 128 : (m + 1) * 128],
                        rhs=ones_sb[:],
                        start=False,
                        stop=True,
                        skip_group_check=True,
                    )
                    y_sb = ppool.tile([128, 512], fp32, tag="y")
                    nc.vector.tensor_copy(y_sb[:], y_ps[:])
                    nc.sync.dma_start(out_t[:, m, nt * 512 : (nt + 1) * 512], y_sb[:])
            pjps_cm.__exit__(None, None, None)

    nc.finalize()
    return nc


def _shard_inputs(x, mask, w_qkv, w_proj, b_proj):
    in_maps = []
    for c in range(NCORES):
        b, g = c // GPB, c % GPB
        heads = [3 * g, 3 * g + 1, 3 * g + 2]
        qk_cols = [h * HD + d for h in heads for d in range(HD)] + [
            C + h * HD + d for h in heads for d in range(HD)
        ]
        v_cols = [2 * C + h * HD + d for h in heads for d in range(HD)]
        mrow = mask[b].astype(np.float32)
        in_maps.append(
            {
                "xT": np.ascontiguousarray(x[b].T).astype(bf),
                "wqk": np.ascontiguousarray(w_qkv[:, qk_cols]).astype(bf),
                "wv": np.ascontiguousarray(w_qkv[:, v_cols]).astype(bf),
                "wp": w_proj.astype(bf),
                "bp": b_proj.reshape(1, C).astype(bf),
                "mb": np.ascontiguousarray(
                    np.where(mrow > 0.5, 0.0, MASK_NEG).astype(np.float32).reshape(KCH, 128).T
                ),
                "mf": np.ascontiguousarray(mrow.reshape(KCH, 128).T),
            }
        )
    return in_maps


def kernel(x, mask, w_qkv, w_proj, b_proj, _trace=False):
    from concourse.bass_utils import run_bass_kernel_spmd

    if "nc" not in _cache:
        _cache["nc"] = _build()
    nc = _cache["nc"]
    in_maps = _shard_inputs(x, mask, w_qkv, w_proj, b_proj)
    res = run_bass_kernel_spmd(nc, in_maps, core_ids=list(range(NCORES)), trace=_trace)
    y = np.empty((B, N, C), dtype=np.float32)
    for b in range(B):
        y[b] = np.asarray(res.results[GPB * b]["out"]).T
    if _trace:
        _cache["last_exec_time_ns"] = res.exec_time_ns
        _cache["last_profile"] = res.profile_json
    return y


# revision 13
# speedup vs baseline: 1.1585x; 1.0566x over previous
"""Trainium2 Bass kernel: 12-head attention (B=2, N=2048, C=768) on 8 NeuronCores.

Sharding: core c -> batch b = c // 4, head-group g = c % 4 (heads 3g..3g+2).
Per core: column-sharded QKV projection, head-sharded attention, AllGather of
normalized attention outputs (channel-major) within each batch's 4-core group,
then the output projection.

Device layouts are "transposed" (channel-major, [C, tokens]) so that
 - the exp mask bias is a per-partition (key) ACT bias,
 - the AllGather concat axis (partitions) is the head/channel axis,
 - the projection consumes the gathered tensor directly as matmul rhs.
The softmax denominator is obtained via an extra all-ones column appended to V
(so P@[V*m, m] yields both the weighted values and the row sums in one pass),
and the division is applied via vector reciprocal + gpsimd partition_broadcast.
"""

import numpy as np
import ml_dtypes

B, N, C = 2, 2048, 768
H, HD = 12, 64
HPG = 3            # heads per core
GPB = 4            # cores (head-groups) per batch
NCORES = 8
SCALE = float(HD) ** -0.5
MASK_NEG = -50.0
KCH = N // 128     # 16 key chunks
DCH = C // 128     # 6 contraction chunks

bf = ml_dtypes.bfloat16

_cache = {}


def _build():
    import concourse.bass as bass
    import concourse.mybir as mybir
    import concourse.tile as tile
    from concourse import bacc

    fp32 = mybir.dt.float32
    bfl = mybir.dt.bfloat16
    EXP = mybir.ActivationFunctionType.Exp
    MULT = mybir.AluOpType.mult

    nc = bacc.Bacc(None, num_devices=NCORES)
    xT = nc.declare_dram_parameter("xT", [C, N], bfl, isOutput=False)
    wqk = nc.declare_dram_parameter("wqk", [C, 2 * HPG * HD], bfl, isOutput=False)
    wv = nc.declare_dram_parameter("wv", [C, HPG * HD], bfl, isOutput=False)
    wp = nc.declare_dram_parameter("wp", [C, C], bfl, isOutput=False)
    bp = nc.declare_dram_parameter("bp", [1, C], bfl, isOutput=False)
    mb = nc.declare_dram_parameter("mb", [128, KCH], fp32, isOutput=False)
    mf = nc.declare_dram_parameter("mf", [128, KCH], fp32, isOutput=False)
    out = nc.declare_dram_parameter("out", [C, 2 * 512], fp32, isOutput=True)

    with tile.TileContext(nc) as tc:
        with (
            tc.tile_pool(name="const", bufs=1) as cpool,
            tc.tile_pool(name="work", bufs=1) as wpool,
            tc.tile_pool(name="pp", bufs=3) as ppool,
            tc.tile_pool(name="dram", bufs=1, space="DRAM") as dpool,
        ):
            # ---------------- input loads ----------------
            xT_sb = cpool.tile([128, DCH, N], bfl, tag="xT")
            xT_r = xT.rearrange("(o p) t -> p o t", p=128)
            for tq in range(4):
                nc.sync.dma_start(
                    xT_sb[:, :, tq * 512 : (tq + 1) * 512],
                    xT_r[:, :, tq * 512 : (tq + 1) * 512],
                )
            wqk_sb = cpool.tile([128, DCH, 2 * HPG * HD], bfl, tag="wqk")
            nc.sync.dma_start(wqk_sb[:], wqk.rearrange("(o p) c -> p o c", p=128))
            wv_sb = cpool.tile([128, DCH, HPG * HD], bfl, tag="wv")
            nc.sync.dma_start(wv_sb[:], wv.rearrange("(o p) c -> p o c", p=128))
            wp_sb = cpool.tile([128, DCH, C], bfl, tag="wp")
            nc.sync.dma_start(wp_sb[:], wp.rearrange("(o p) c -> p o c", p=128))
            bp_sb = cpool.tile([1, C], bfl, tag="bp")
            nc.sync.dma_start(bp_sb[:], bp[:])
            ones_sb = cpool.tile([1, 512], bfl, tag="ones")
            nc.vector.memset(ones_sb[:], 1.0)
            mb_sb = cpool.tile([128, KCH], fp32, tag="mb")
            nc.sync.dma_start(mb_sb[:], mb[:])
            mf_sb = cpool.tile([128, KCH], fp32, tag="mf")
            nc.sync.dma_start(mf_sb[:], mf[:])

            qT = wpool.tile([64, HPG, N], bfl, tag="qT")
            kT = wpool.tile([64, HPG, N], bfl, tag="kT")
            V3 = wpool.tile([128, KCH, HPG, HD + 1], bfl, tag="V3")

            qkps_cm = tc.tile_pool(name="qkps", bufs=2, space="PSUM")
            vps_cm = tc.tile_pool(name="vps", bufs=2, space="PSUM")
            qkps = qkps_cm.__enter__(); vps = vps_cm.__enter__()

            # ---------------- V projection (token-major) + mask fold ----------------
            for i in range(KCH):
                v_t = vps.tile([128, HPG * HD], fp32, tag="v")
                for kk in range(DCH):
                    nc.tensor.matmul(
                        v_t[:],
                        lhsT=xT_sb[:, kk, i * 128 : (i + 1) * 128],
                        rhs=wv_sb[:, kk, :],
                        start=(kk == 0),
                        stop=(kk == DCH - 1),
                    )
                nc.vector.tensor_scalar_mul(
                    V3[:, i, :, 0:HD],
                    v_t[:].rearrange("p (h d) -> p h d", h=HPG),
                    mf_sb[:, i : i + 1],
                )
                nc.vector.tensor_copy(
                    V3[:, i, :, HD],
                    mf_sb[:, i : i + 1].to_broadcast((128, HPG)),
                )

            # ---------------- QK^T projection (channel-major) ----------------
            # wqk columns: [q_h0 q_h1 q_h2 k_h0 k_h1 k_h2], 64 each.
            dests = [(qT, 0), (kT, 0), (qT, 1), (kT, 1), (qT, 2), (kT, 2)]
            for m in range(3):
                for nt in range(2):
                    qk_t = qkps.tile([128, N // 2], fp32, tag="qk")
                    for n2 in range(2):
                        for kk in range(DCH):
                            nc.tensor.matmul(
                                qk_t[:, n2 * 512 : (n2 + 1) * 512],
                                lhsT=wqk_sb[:, kk, m * 128 : (m + 1) * 128],
                                rhs=xT_sb[
                                    :, kk, nt * 1024 + n2 * 512 : nt * 1024 + (n2 + 1) * 512
                                ],
                                start=(kk == 0),
                                stop=(kk == DCH - 1),
                            )
                    for half in range(2):
                        dtile, j = dests[m * 2 + half]
                        nc.vector.tensor_copy(
                            dtile[:, j, nt * 1024 : (nt + 1) * 1024],
                            qk_t[half * 64 : (half + 1) * 64, :],
                        )

            vps_cm.__exit__(None, None, None); qkps_cm.__exit__(None, None, None)

            # ---------------- attention per head ----------------
            sps_cm = tc.tile_pool(name="sps", bufs=2, space="PSUM")
            ops_cm = tc.tile_pool(name="ops", bufs=1, space="PSUM")
            sps = sps_cm.__enter__(); ops = ops_cm.__enter__()
            OnA = wpool.tile([128, N], bfl, tag="OnA")
            OnB = wpool.tile([64, N], bfl, tag="OnB")
            for h in range(HPG):
                o_t = ops.tile([HD + 1, N], fp32, tag="o")
                for i in range(KCH):
                    for qh in range(2):
                        s_t = sps.tile([128, N // 2], fp32, tag="s")
                        for n2 in range(2):
                            nc.tensor.matmul(
                                s_t[:, n2 * 512 : (n2 + 1) * 512],
                                lhsT=kT[:, h, i * 128 : (i + 1) * 128],
                                rhs=qT[:, h, qh * 1024 + n2 * 512 : qh * 1024 + (n2 + 1) * 512],
                                start=True,
                                stop=True,
                            )
                        p_t = ppool.tile([128, N // 2], bfl, tag="p")
                        nc.scalar.activation(
                            p_t[:], s_t[:], EXP, bias=mb_sb[:, i : i + 1], scale=SCALE
                        )
                        for n2 in range(2):
                            qt = qh * 2 + n2
                            nc.tensor.matmul(
                                o_t[:, qt * 512 : (qt + 1) * 512],
                                lhsT=V3[:, i, h, :],
                                rhs=p_t[:, n2 * 512 : (n2 + 1) * 512],
                                start=(i == 0),
                                stop=(i == KCH - 1),
                            )
                # normalization: rows 0..63 = unnormalized O^T, row 64 = sums
                sums = wpool.tile([1, N], fp32, tag="sums")
                nc.scalar.copy(sums[:], o_t[HD : HD + 1, :])
                o_raw = wpool.tile([HD, N], fp32, tag=f"oraw{h}")
                nc.vector.tensor_copy(o_raw[:], o_t[0:HD, :])
                rbraw = wpool.tile([HD, N], fp32, tag="rbraw")
                nc.gpsimd.partition_broadcast(rbraw[:], sums[:])
                rb = wpool.tile([HD, N], fp32, tag="rb")
                nc.vector.reciprocal_approx_fast(rb[:], rbraw[:])
                dst = OnA[h * 64 : (h + 1) * 64, :] if h < 2 else OnB[:, :]
                nc.vector.tensor_tensor(dst, o_raw[:], rb[:], MULT)

            ops_cm.__exit__(None, None, None); sps_cm.__exit__(None, None, None)

            # -- 8-core AllToAll: block d = my [192ch, tok slice d%4] --
            # receiver c gets block r = rank r's channels at token slice c%4;
            # rows 0..768 = batch-0 channels, 768..1536 = batch-1 channels.
            ag_in = nc.dram_tensor("ag_in", [NCORES * HPG * HD, 512], bfl)
            ag_out = nc.dram_tensor("ag_out", [NCORES * HPG * HD, 512], bfl)
            for j in range(NCORES):
                g = j % GPB
                nc.sync.dma_start(
                    ag_in[j * 192 : j * 192 + 128, :], OnA[:, g * 512 : (g + 1) * 512]
                )
                nc.sync.dma_start(
                    ag_in[j * 192 + 128 : (j + 1) * 192, :], OnB[:, g * 512 : (g + 1) * 512]
                )
            nc.gpsimd.collective_compute(
                "AllToAll",
                mybir.AluOpType.bypass,
                replica_groups=[[0, 1, 2, 3, 4, 5, 6, 7]],
                ins=[ag_in[:].opt()],
                outs=[ag_out[:].opt()],
            )

            # ---------------- output projection ----------------
            pjps_cm = tc.tile_pool(name="pjps", bufs=2, space="PSUM")
            pjps = pjps_cm.__enter__()
            ag_out_t = ag_out.rearrange("(o p) t -> p o t", p=128)
            out_t = out.rearrange("(o p) t -> p o t", p=128)
            at_sb = ppool.tile([128, 2 * DCH, 512], bfl, tag="at")
            nc.sync.dma_start(at_sb[:], ag_out_t[:])
            for b2 in range(2):
                for m in range(DCH):
                    y_ps = pjps.tile([128, 512], fp32, tag="yps")
                    for kk in range(DCH):
                        nc.tensor.matmul(
                            y_ps[:],
                            lhsT=wp_sb[:, kk, m * 128 : (m + 1) * 128],
                            rhs=at_sb[:, b2 * DCH + kk, :],
                            start=(kk == 0),
                            stop=False,
                        )
                    nc.tensor.matmul(
                        y_ps[:],
                        lhsT=bp_sb[:, m * 128 : (m + 1) * 128],
                        rhs=ones_sb[:],
                        start=False,
                        stop=True,
                        skip_group_check=True,
                    )
                    y_sb = ppool.tile([128, 512], fp32, tag="y")
                    nc.vector.tensor_copy(y_sb[:], y_ps[:])
                    nc.sync.dma_start(out_t[:, m, b2 * 512 : (b2 + 1) * 512], y_sb[:])
            pjps_cm.__exit__(None, None, None)

    nc.finalize()
    return nc


def _shard_inputs(x, mask, w_qkv, w_proj, b_proj):
    in_maps = []
    for c in range(NCORES):
        b, g = c // GPB, c % GPB
        heads = [3 * g, 3 * g + 1, 3 * g + 2]
        qk_cols = [
            base + h * HD + d
            for h in heads
            for base in (0, C)
            for d in range(HD)
        ]
        v_cols = [2 * C + h * HD + d for h in heads for d in range(HD)]
        mrow = mask[b].astype(np.float32)
        in_maps.append(
            {
                "xT": np.ascontiguousarray(x[b].T).astype(bf),
                "wqk": np.ascontiguousarray(w_qkv[:, qk_cols]).astype(bf),
                "wv": np.ascontiguousarray(w_qkv[:, v_cols]).astype(bf),
                "wp": w_proj.astype(bf),
                "bp": b_proj.reshape(1, C).astype(bf),
                "mb": np.ascontiguousarray(
                    np.where(mrow > 0.5, 0.0, MASK_NEG).astype(np.float32).reshape(KCH, 128).T
                ),
                "mf": np.ascontiguousarray(mrow.reshape(KCH, 128).T),
            }
        )
    return in_maps


def kernel(x, mask, w_qkv, w_proj, b_proj, _trace=False):
    from concourse.bass_utils import run_bass_kernel_spmd

    if "nc" not in _cache:
        _cache["nc"] = _build()
    nc = _cache["nc"]
    in_maps = _shard_inputs(x, mask, w_qkv, w_proj, b_proj)
    res = run_bass_kernel_spmd(nc, in_maps, core_ids=list(range(NCORES)), trace=_trace)
    y = np.empty((B, N, C), dtype=np.float32)
    for c in range(NCORES):
        b, g = c // GPB, c % GPB
        y[b, g * 512 : (g + 1) * 512] = np.asarray(
            res.results[c]["out"][:, b * 512 : (b + 1) * 512]
        ).T
    if _trace:
        _cache["last_exec_time_ns"] = res.exec_time_ns
        _cache["last_profile"] = res.profile_json
    return y
